# revision 22
# baseline (speedup 1.0000x reference)
"""Trainium2 Bass kernel for GCE-TAGNN session recommendation model.

Strategy (v2):
  - Vocab axis (10240 = 8*1280) sharded across 8 cores for the global sparse
    aggregation and the candidate transforms; session path data-parallel
    (8 sessions/core); `final` all-gathered in bf16 so every core runs the
    target attention for the full batch over its candidate shard.
  - Global aggregation: edges pre-sorted by row into 128-row windows; the
    one-hot scatter matrices (one-hot(row_rel) * w) are built on the host and
    streamed as bf16, so phase A is pure DMA + chained bf16 matmuls.
  - Target attention in [session-position, candidate] layout:
      ts[bl, n] = final[bl]. (W_t cand[n]),  E = exp(ts),  g[bl, n] = final.d1
      scores[b, n] = (S^T (E*g))[b,n] / (S^T E)[b,n] + last.d2 + sglob.d3
    where S[bl, b] is a host-built 0/1 matrix marking valid (non-pad)
    positions of session b.  Both softmax reductions are matmuls, so the
    inner loop is 4 matmuls + 1 exp + 1 multiply per 128-position chunk,
    software-pipelined to keep the PE at full clock.
"""

import sys

sys.path.insert(0, "/opt/trn_rl_repo")

import math

import ml_dtypes
import numpy as np

import concourse.bass as bass
import concourse.mybir as mybir
import concourse.tile as tile
from concourse import bacc
from concourse.bass import IndirectOffsetOnAxis
from concourse.bass_utils import run_bass_kernel_spmd

F32 = mybir.dt.float32
F32R = mybir.dt.float32r
BF16 = mybir.dt.bfloat16
I32 = mybir.dt.int32
AX = mybir.AxisListType
ALU = mybir.AluOpType
ACT = mybir.ActivationFunctionType

NC = 8          # cores
B = 64          # batch
L = 50          # session length
H = 128         # hidden
NH = 8          # heads
NIT = 10000     # item vocab
NPAD = NC * 1280  # padded vocab for candidate sharding
NS = 1280       # candidate shard per core
BLOC = B // NC  # sessions per core
RL = BLOC * L   # 400 rows per core
BL = B * L      # 3200 positions total
NCH = BL // H   # 25 position chunks
WIN = 128       # agg row window
NWIN = NS // WIN  # 10 windows per core
NSPL = [(0, 512), (512, 512), (1024, 256)]  # candidate splits (PSUM banks)


def _f32r(ap):
    return ap


_NC_CACHE = {}


def build_nc(T, lens):
    """Build the single-NEFF SPMD program. T = edge tiles per window;
    lens = per-session valid lengths (compile-baked packing offsets)."""
    offs = [0]
    for lb in lens:
        offs.append(offs[-1] + int(lb))
    NVP = offs[-1]                      # total valid positions
    NCHP = (NVP + H - 1) // H           # packed chunks
    BLP = NCHP * H
    nc = bacc.Bacc(None, target_bir_lowering=False)

    def inp(name, shape, dtype=F32):
        return nc.dram_tensor(name, shape, dtype, kind="ExternalInput")

    # ---- replicated weights/constants ----
    embf = inp("embf", [NIT, H])
    posemb = inp("posemb", [200, H])
    idn = inp("idn", [H, H])
    idnb = inp("idnb", [H, H], BF16)
    blockdiag = inp("blockdiag", [H, NH])
    w_lin_inT = inp("w_lin_inT", [H, H])
    w_lin_outT = inp("w_lin_outT", [H, H])
    b_lin_in = inp("b_lin_in", [H, 1])
    b_lin_out = inp("b_lin_out", [H, 1])
    w_ihT = inp("w_ihT", [2 * H, 3 * H])
    w_hhT = inp("w_hhT", [H, 3 * H])
    b_ih = inp("b_ih", [3 * H, 1])
    b_hh = inp("b_hh", [3 * H, 1])
    in_projT = inp("in_projT", [H, 3 * H])
    in_projb = inp("in_projb", [3 * H, 1])
    out_projT = inp("out_projT", [H, H])
    out_projb = inp("out_projb", [H, 1])
    gWTb = inp("gWTb", [H, H], BF16)
    gb = inp("gb", [H, 1])
    w3b = inp("w3b", [H, 3 * H], BF16)
    wtTb = inp("wtTb", [H, H], BF16)
    ssel = inp("ssel", [H, NCHP, B], BF16)
    # ---- per-core ----
    adjT = inp("adjT", [BLOC, L, L])
    itemsx = inp("itemsx", [512, 1], I32)
    revx = inp("revx", [512, 1], I32)
    attmaskr = inp("attmaskr", [NH, RL])
    lastselr = inp("lastselr", [H, RL])
    candTb = inp("candTb", [H, NS], BF16)
    eemb = inp("eemb", [H, NWIN * T, H], BF16)
    sww = inp("sww", [H, NWIN * T, WIN], BF16)

    scores_out = nc.dram_tensor("scores", [B, NS], F32, kind="ExternalOutput")

    with tile.TileContext(nc) as tc:
        with (
            tc.tile_pool(name="cst", bufs=1) as cst,
            tc.tile_pool(name="wk", bufs=3) as wk,
            tc.tile_pool(name="pp", bufs=8, space="PSUM") as pp,
            tc.tile_pool(name="dr", bufs=1, space="DRAM") as dr,
        ):
            def psum(shape, tag="ts", nbuf=2, dtype=F32):
                return pp.tile(shape, dtype, tag=tag, name=tag, bufs=nbuf)

            # ---------- load constants into SBUF ----------
            def load(name, src, shape=None, dtype=F32):
                t = cst.tile(shape if shape is not None else src.shape, dtype, name=name)
                nc.sync.dma_start(t[:], src[:])
                return t

            # small consts first (they gate early compute); big slabs after
            NT = NWIN * T
            items_sb = cst.tile([H, 4], I32, name="items_sb")
            nc.sync.dma_start(items_sb[:], itemsx.rearrange("(t p) o -> p (t o)", p=H))
            rev_sb = cst.tile([H, 4], I32, name="rev_sb")
            nc.sync.dma_start(rev_sb[:], revx.rearrange("(t p) o -> p (t o)", p=H))
            gWT_sb = load("gWT_sb", gWTb, dtype=BF16)
            gb_sb = load("gb_sb", gb)
            idnb_sb = load("idnb_sb", idnb, dtype=BF16)
            candT_sb = load("candT_sb", candTb, dtype=BF16)
            w3_sb = load("w3_sb", w3b, dtype=BF16)
            wtT_sb = load("wtT_sb", wtTb, dtype=BF16)
            idn_sb = load("idn_sb", idn)
            eemb_sb = cst.tile([H, NT, H], BF16, name="eemb_sb")
            sww_sb = cst.tile([H, NT, WIN], BF16, name="sww_sb")
            bnd = [round(k * NT / 8) for k in range(9)]
            for k in range(8):
                s, e = bnd[k], bnd[k + 1]
                nc.sync.dma_start(eemb_sb[:, s:e, :], eemb[:, s:e, :])
                nc.sync.dma_start(sww_sb[:, s:e, :], sww[:, s:e, :])
            bd_sb = load("bd_sb", blockdiag)
            linT_sb = load("linT_sb", w_lin_inT)
            loutT_sb = load("loutT_sb", w_lin_outT)
            blin_sb = load("blin_sb", b_lin_in)
            blout_sb = load("blout_sb", b_lin_out)
            wih_sb = cst.tile([H, 2, 3 * H], F32, name="wih_sb")
            nc.sync.dma_start(wih_sb[:], w_ihT.rearrange("(a p) j -> p a j", p=H))
            whh_sb = load("whh_sb", w_hhT)
            bih_sb = cst.tile([H, 3], F32, name="bih_sb")
            bhh_sb = cst.tile([H, 3], F32, name="bhh_sb")
            nc.sync.dma_start(bih_sb[:], b_ih.rearrange("(g p) o -> p (g o)", p=H))
            nc.sync.dma_start(bhh_sb[:], b_hh.rearrange("(g p) o -> p (g o)", p=H))
            prjT_sb = load("prjT_sb", in_projT)
            prjb_sb = cst.tile([H, 3], F32, name="prjb_sb")
            nc.sync.dma_start(prjb_sb[:], in_projb.rearrange("(g p) o -> p (g o)", p=H))
            oprjT_sb = load("oprjT_sb", out_projT)
            oprjb_sb = load("oprjb_sb", out_projb)
            ssel_sb = load("ssel_sb", ssel, dtype=BF16)
            am_sb = load("am_sb", attmaskr)
            ls_sb = load("ls_sb", lastselr)

            # DRAM bounce buffers for collectives
            hg_shard = dr.tile([NS, H], BF16, name="hg_shard")
            hg_full = dr.tile([NC * NS, H], BF16, addr_space="Shared", name="hg_full")
            f_shard = dr.tile([H, RL], BF16, name="f_shard")
            f_full = dr.tile([NC * H, RL], BF16, addr_space="Shared", name="f_full")
            ls_shard = dr.tile([H, 2 * NH], BF16, name="ls_shard")
            ls_full = dr.tile([NC * H, 2 * NH], BF16, addr_space="Shared",
                              name="ls_full")

            # issue h0/pos gathers early (gpsimd is idle during phase A)
            gh0 = cst.tile([H, 4, H], F32, name="gh0")
            gpo = cst.tile([H, 4, H], F32, name="gpo")
            for t in range(4):
                nc.gpsimd.indirect_dma_start(
                    out=gh0[:, t, :], out_offset=None, in_=embf[:, :],
                    in_offset=IndirectOffsetOnAxis(ap=items_sb[:, t:t + 1], axis=0))
                nc.gpsimd.indirect_dma_start(
                    out=gpo[:, t, :], out_offset=None, in_=posemb[:, :],
                    in_offset=IndirectOffsetOnAxis(ap=rev_sb[:, t:t + 1], axis=0))

            # =======================================================
            # Phase A: global GNN aggregation (vocab shard, 10 windows)
            # agg[h, r] = sum_edges w * emb[col, h] via one-hot matmuls
            # =======================================================
            aggT = cst.tile([H, NS], BF16, name="aggT")
            for w in range(NWIN):
                agg_ps = psum([H, WIN], tag="ts")
                for t in range(T):
                    j = w * T + t
                    nc.tensor.matmul(agg_ps[:], eemb_sb[:, j, :], sww_sb[:, j, :],
                                     start=(t == 0), stop=(t == T - 1))
                nc.vector.tensor_copy(aggT[:, w * WIN:(w + 1) * WIN], agg_ps[:])
            # hgT = relu(gW @ agg + gb) in bf16
            hgT = cst.tile([H, NS], BF16, name="hgT")
            for off, w in NSPL:
                ps = psum([H, w], tag="gg")
                nc.tensor.matmul(ps[:], gWT_sb[:], aggT[:, off:off + w])
                nc.scalar.activation(hgT[:, off:off + w], ps[:], ACT.Relu,
                                     bias=gb_sb[:, :1])
            # transpose to row-major [1280, 128] and store for all-gather
            hg_rm = cst.tile([H, NWIN, H], BF16, name="hg_rm")
            for k in range(NWIN):
                ps_b = psum([H, H], tag="num", dtype=BF16)
                nc.tensor.transpose(ps_b[:], hgT[:, k * H:(k + 1) * H], idnb_sb[:])
                nc.vector.tensor_copy(hg_rm[:, k, :], ps_b[:])
            nc.sync.dma_start(hg_shard.rearrange("(k p) h -> p k h", p=H), hg_rm[:])
            nc.gpsimd.collective_compute(
                "AllGather", ALU.bypass, replica_groups=[list(range(NC))],
                ins=[hg_shard[:].opt()], outs=[hg_full[:].opt()])

            # =======================================================
            # Phase C: candidate transforms (runs in AG1's shadow)
            # cT[j][r, n] = d[n, j*128+r] with d = cand @ w3_W; trT = W_t@cand^T
            # =======================================================
            cT = [cst.tile([H, NS], BF16, name=f"c{j}T") for j in range(3)]
            trT = cst.tile([H, NS], BF16, name="trT")
            for j in range(3):
                for off, w in NSPL:
                    ps = psum([H, w], tag="ts")
                    nc.tensor.matmul(
                        ps[:], w3_sb[:, j * H:(j + 1) * H],
                        candT_sb[:, off:off + w])
                    nc.vector.tensor_copy(cT[j][:, off:off + w], ps[:])
            for off, w in NSPL:
                ps = psum([H, w], tag="gg")
                nc.tensor.matmul(ps[:], wtT_sb[:], candT_sb[:, off:off + w])
                nc.vector.tensor_copy(trT[:, off:off + w], ps[:])

            # =======================================================
            # Phase B: session path (8 local sessions)
            # =======================================================
            def gather_T(dst, table, idx_sb, tag, dtype=F32):
                """gather rows table[idx] -> transpose -> dst [128, 512]."""
                for t in range(4):
                    g = wk.tile([H, H], dtype, tag=tag)
                    nc.gpsimd.indirect_dma_start(
                        out=g[:], out_offset=None, in_=table[:, :],
                        in_offset=IndirectOffsetOnAxis(ap=idx_sb[:, t:t + 1], axis=0))
                    if dtype == BF16:
                        ps_g2 = psum([H, H], tag="num", dtype=BF16)
                        nc.tensor.transpose(ps_g2[:], g[:], idnb_sb[:])
                        nc.vector.tensor_copy(dst[:, t * H:(t + 1) * H], ps_g2[:])
                    else:
                        ps = psum([H, H], tag="den")
                        nc.tensor.transpose(ps[:], g[:], idn_sb[:])
                        nc.vector.tensor_copy(dst[:, t * H:(t + 1) * H], ps[:])

            def transpose_T(dst, gsrc):
                for t in range(4):
                    ps = psum([H, H], tag="den")
                    nc.tensor.transpose(ps[:], gsrc[:, t, :], idn_sb[:])
                    nc.vector.tensor_copy(dst[:, t * H:(t + 1) * H], ps[:])

            h0T = cst.tile([H, 512], F32, name="h0T")
            transpose_T(h0T, gh0)

            # Y = lin(h);  inp = adj @ Y   (per session)
            yinT = cst.tile([H, RL], F32, name="yinT")
            youtT = cst.tile([H, RL], F32, name="youtT")
            ps = psum([H, RL], tag="ts")
            nc.tensor.matmul(ps[:], _f32r(linT_sb[:]), _f32r(h0T[:, :RL]))
            nc.vector.tensor_scalar_add(yinT[:], ps[:], blin_sb[:, :1])
            ps = psum([H, RL], tag="gg")
            nc.tensor.matmul(ps[:], _f32r(loutT_sb[:]), _f32r(h0T[:, :RL]))
            nc.vector.tensor_scalar_add(youtT[:], ps[:], blout_sb[:, :1])

            iinT = cst.tile([H, RL], F32, name="iinT")
            ioutT = cst.tile([H, RL], F32, name="ioutT")
            for b in range(BLOC):
                at = wk.tile([L, L], F32, tag="at")
                nc.sync.dma_start(at[:], adjT[b])
                for yT, dst in ((yinT, iinT), (youtT, ioutT)):
                    ps_t = psum([L, H], tag="den")
                    nc.tensor.transpose(ps_t[:], yT[:, b * L:(b + 1) * L], idn_sb[:])
                    yb = wk.tile([L, H], F32, tag="yb")
                    nc.vector.tensor_copy(yb[:], ps_t[:])
                    ps_i = psum([H, L], tag="num")
                    nc.tensor.matmul(ps_i[:], yb[:], at[:])
                    nc.vector.tensor_copy(dst[:, b * L:(b + 1) * L], ps_i[:])

            # GRU cell (feature-major)
            combR = cst.tile([H, 2], F32, name="combR")
            nc.vector.tensor_add(combR[:, 0:1], bih_sb[:, 0:1], bhh_sb[:, 0:1])
            nc.vector.tensor_add(combR[:, 1:2], bih_sb[:, 1:2], bhh_sb[:, 1:2])
            gates = []
            for g in range(2):  # r, z
                ps_g = psum([H, RL], tag="ts" if g == 0 else "gg")
                nc.tensor.matmul(ps_g[:], _f32r(wih_sb[:, 0, g * H:(g + 1) * H]),
                                 _f32r(iinT[:]), start=True, stop=False)
                nc.tensor.matmul(ps_g[:], _f32r(wih_sb[:, 1, g * H:(g + 1) * H]),
                                 _f32r(ioutT[:]), start=False, stop=False)
                nc.tensor.matmul(ps_g[:], _f32r(whh_sb[:, g * H:(g + 1) * H]),
                                 _f32r(h0T[:, :RL]), start=False, stop=True)
                gt = cst.tile([H, RL], F32, name=f"gate{g}")
                nc.scalar.activation(gt[:], ps_g[:], ACT.Sigmoid, bias=combR[:, g:g + 1])
                gates.append(gt)
            rT, zT = gates
            ps_in = psum([H, RL], tag="ts")
            nc.tensor.matmul(ps_in[:], _f32r(wih_sb[:, 0, 2 * H:3 * H]), _f32r(iinT[:]),
                             start=True, stop=False)
            nc.tensor.matmul(ps_in[:], _f32r(wih_sb[:, 1, 2 * H:3 * H]), _f32r(ioutT[:]),
                             start=False, stop=True)
            ps_hn = psum([H, RL], tag="gg")
            nc.tensor.matmul(ps_hn[:], _f32r(whh_sb[:, 2 * H:3 * H]), _f32r(h0T[:, :RL]))
            rhn = cst.tile([H, RL], F32, name="rhn")
            nc.vector.scalar_tensor_tensor(
                out=rhn[:], in0=ps_hn[:], scalar=bhh_sb[:, 2:3], in1=rT[:],
                op0=ALU.add, op1=ALU.mult)
            tmp_n = cst.tile([H, RL], F32, name="tmp_n")
            nc.vector.tensor_add(tmp_n[:], ps_in[:], rhn[:])
            nT = cst.tile([H, RL], F32, name="nT")
            nc.scalar.activation(nT[:], tmp_n[:], ACT.Tanh, bias=bih_sb[:, 2:3])
            diff = cst.tile([H, RL], F32, name="diff")
            nc.vector.tensor_sub(diff[:], h0T[:, :RL], nT[:])
            zd = cst.tile([H, RL], F32, name="zd")
            nc.vector.tensor_mul(zd[:], zT[:], diff[:])
            h1T = cst.tile([H, RL], F32, name="h1T")
            nc.vector.tensor_add(h1T[:], nT[:], zd[:])

            # rich = hg[items] + h1; final = rich + pos_emb[rev]
            sgT = cst.tile([H, 512], BF16, name="sgT")
            gather_T(sgT, hg_full, items_sb, "gsg", dtype=BF16)
            poT = cst.tile([H, 512], F32, name="poT")
            transpose_T(poT, gpo)
            richT = cst.tile([H, RL], F32, name="richT")
            nc.vector.tensor_add(richT[:], h1T[:], sgT[:, :RL])
            # ship final (bf16) for all-gather ASAP; rest overlaps the collective
            finB = cst.tile([H, RL], BF16, name="finB")
            nc.vector.tensor_add(finB[:], richT[:], poT[:, :RL])
            nc.sync.dma_start(f_shard[:], finB[:])
            nc.gpsimd.collective_compute(
                "AllGather", ALU.bypass, replica_groups=[list(range(NC))],
                ins=[f_shard[:].opt()], outs=[f_full[:].opt()])
            finT = cst.tile([H, RL], F32, name="finT")
            nc.vector.tensor_add(finT[:], richT[:], poT[:, :RL])


            # last[b] = final[b, len_b - 1]  (one-hot selection + reduce)
            lsel = cst.tile([H, RL], F32, name="lsel")
            nc.vector.tensor_mul(lsel[:], finT[:], ls_sb[:])
            lastT = cst.tile([H, NH], F32, name="lastT")
            nc.vector.reduce_sum(lastT[:], lsel[:].rearrange("p (b l) -> p b l", b=BLOC),
                                 axis=AX.X)
            # ---- multi-head attention (q = last, kv = final) ----
            qT = cst.tile([H, NH], F32, name="qT")
            ps_q = psum([H, NH], tag="den")
            nc.tensor.matmul(ps_q[:], _f32r(prjT_sb[:, 0:H]), _f32r(lastT[:]))
            nc.vector.tensor_scalar_add(qT[:], ps_q[:], prjb_sb[:, 0:1])
            kT = cst.tile([H, RL], F32, name="kT")
            ps_k = psum([H, RL], tag="ts")
            nc.tensor.matmul(ps_k[:], _f32r(prjT_sb[:, H:2 * H]), _f32r(finT[:]))
            nc.vector.tensor_scalar_add(kT[:], ps_k[:], prjb_sb[:, 1:2])
            vT = cst.tile([H, RL], F32, name="vT")
            ps_v = psum([H, RL], tag="gg")
            nc.tensor.matmul(ps_v[:], _f32r(prjT_sb[:, 2 * H:3 * H]), _f32r(finT[:]))
            nc.vector.tensor_scalar_add(vT[:], ps_v[:], prjb_sb[:, 2:3])

            ctxT = cst.tile([H, NH], F32, name="ctxT")
            for b in range(BLOC):
                qb = wk.tile([H, NH], F32, tag="qb")
                nc.vector.tensor_mul(qb[:], qT[:, b:b + 1].to_broadcast([H, NH]), bd_sb[:])
                ps_a = psum([NH, L], tag="ts")
                nc.tensor.matmul(ps_a[:], qb[:], kT[:, b * L:(b + 1) * L])
                attm = wk.tile([NH, L], F32, tag="attm")
                nc.vector.tensor_add(attm[:], ps_a[:], am_sb[:, b * L:(b + 1) * L])
                negmax = wk.tile([NH, 1], F32, tag="negmax")
                nc.vector.tensor_reduce(negmax[:], attm[:], axis=AX.X, op=ALU.max,
                                        negate=True)
                attE = wk.tile([NH, L], F32, tag="attE")
                den_a = wk.tile([NH, 1], F32, tag="den_a")
                nc.scalar.activation(attE[:], attm[:], ACT.Exp, bias=negmax[:, :1],
                                     accum_out=den_a[:, :1])
                rec_a = wk.tile([NH, 1], F32, tag="rec_a")
                nc.vector.reciprocal(rec_a[:], den_a[:])
                attw = wk.tile([NH, L], F32, tag="attw")
                nc.vector.tensor_scalar_mul(attw[:], attE[:], rec_a[:, :1])
                ps_wt = psum([L, NH], tag="gg")
                nc.tensor.transpose(ps_wt[:], attw[:], idn_sb[:NH, :NH])
                awT = wk.tile([L, NH], F32, tag="awT")
                nc.vector.tensor_copy(awT[:], ps_wt[:])
                ps_vt = psum([L, H], tag="den")
                nc.tensor.transpose(ps_vt[:], vT[:, b * L:(b + 1) * L], idn_sb[:])
                vb = wk.tile([L, H], F32, tag="vb")
                nc.vector.tensor_copy(vb[:], ps_vt[:])
                ps_o = psum([H, NH], tag="num")
                nc.tensor.matmul(ps_o[:], vb[:], awT[:])
                o2 = wk.tile([H, NH], F32, tag="o2")
                nc.vector.tensor_mul(o2[:], ps_o[:], bd_sb[:])
                nc.vector.reduce_sum(ctxT[:, b:b + 1], o2[:], axis=AX.X)

            sgloT = cst.tile([H, NH], F32, name="sgloT")
            ps_sg = psum([H, NH], tag="den")
            nc.tensor.matmul(ps_sg[:], _f32r(oprjT_sb[:]), _f32r(ctxT[:]))
            nc.vector.tensor_scalar_add(sgloT[:], ps_sg[:], oprjb_sb[:, :1])

            lsB = cst.tile([H, 2 * NH], BF16, name="lsB")
            nc.vector.tensor_copy(lsB[:, 0:NH], lastT[:])
            nc.vector.tensor_copy(lsB[:, NH:2 * NH], sgloT[:])
            nc.sync.dma_start(ls_shard[:], lsB[:])
            nc.gpsimd.collective_compute(
                "AllGather", ALU.bypass, replica_groups=[list(range(NC))],
                ins=[ls_shard[:].opt()], outs=[ls_full[:].opt()])

            fullT = cst.tile([H, BL], BF16, name="fullT")
            ffv = f_full.rearrange("(c p) r -> p c r", p=H)
            for k in range(4):
                nc.sync.dma_start(
                    fullT[:].rearrange("p (c r) -> p c r", c=NC)[:, 2 * k:2 * k + 2, :],
                    ffv[:, 2 * k:2 * k + 2, :])
            # pack valid session prefixes contiguously (offsets compile-baked)
            fullP = cst.tile([H, BLP], BF16, name="fullP")
            if BLP > NVP:
                nc.vector.memset(fullP[:, NVP:BLP], 0)
            for b in range(B):
                nc.vector.tensor_copy(fullP[:, offs[b]:offs[b + 1]],
                                      fullT[:, b * L:b * L + int(lens[b])])

            lsv = ls_full.rearrange("(c p) x -> p c x", p=H)
            lastF = cst.tile([H, B], BF16, name="lastF")
            sglF = cst.tile([H, B], BF16, name="sglF")
            nc.sync.dma_start(lastF[:].rearrange("p (c x) -> p c x", c=NC),
                              lsv[:, :, 0:NH])
            nc.sync.dma_start(sglF[:].rearrange("p (c x) -> p c x", c=NC),
                              lsv[:, :, NH:2 * NH])

            # =======================================================
            # Phase D: target attention, [position, candidate] layout
            # =======================================================
            def d_main(noff, nw):
                ps_den = psum([B, nw], tag="den")
                ps_num = psum([B, nw], tag="num")
                pend = []
                for c in range(NCHP):
                    ps_ts = psum([H, nw], tag="ts")
                    nc.tensor.matmul(ps_ts[:], fullP[:, c * H:(c + 1) * H],
                                     trT[:, noff:noff + nw])
                    ps_g = psum([H, nw], tag="gg")
                    nc.tensor.matmul(ps_g[:], fullP[:, c * H:(c + 1) * H],
                                     cT[0][:, noff:noff + nw])
                    if len(pend) >= 2:
                        eP, pP, cP = pend.pop(0)
                        nc.tensor.matmul(ps_den[:], ssel_sb[:, cP, :], eP[:],
                                         start=(cP == 0), stop=False)
                        nc.tensor.matmul(ps_num[:], ssel_sb[:, cP, :], pP[:],
                                         start=(cP == 0), stop=False)
                    eT = wk.tile([H, nw], BF16, tag="eT", bufs=4)
                    nc.scalar.activation(eT[:], ps_ts[:], ACT.Exp)
                    pT = wk.tile([H, nw], BF16, tag="pT", bufs=4)
                    nc.vector.tensor_mul(pT[:], eT[:], ps_g[:])
                    pend.append((eT, pT, c))
                for eP, pP, cP in pend:
                    nc.tensor.matmul(ps_den[:], ssel_sb[:, cP, :], eP[:],
                                     start=False, stop=(cP == NCHP - 1))
                    nc.tensor.matmul(ps_num[:], ssel_sb[:, cP, :], pP[:],
                                     start=False, stop=(cP == NCHP - 1))
                return ps_den, ps_num

            def d_epilogue(noff, nw, ps_den, ps_num):
                # scores = num/den + last.d2 + sglob.d3
                rec = wk.tile([B, nw], F32, tag="rec", bufs=2)
                nc.vector.reciprocal(rec[:], ps_den[:])
                ps23 = psum([B, nw], tag="ts")
                nc.tensor.matmul(ps23[:], lastF[:], cT[1][:, noff:noff + nw],
                                 start=True, stop=False)
                nc.tensor.matmul(ps23[:], sglF[:], cT[2][:, noff:noff + nw],
                                 start=False, stop=True)
                t1 = wk.tile([B, nw], F32, tag="t1", bufs=2)
                nc.vector.tensor_mul(t1[:], ps_num[:], rec[:])
                outT = wk.tile([B, nw], F32, tag="outT", bufs=2)
                nc.vector.tensor_add(outT[:], t1[:], ps23[:])
                nc.sync.dma_start(scores_out[:, noff:noff + nw], outT[:])

            pending_epi = None
            for noff, nw in NSPL:
                dn = d_main(noff, nw)
                if pending_epi is not None:
                    d_epilogue(*pending_epi)
                pending_epi = (noff, nw, *dn)
            d_epilogue(*pending_epi)

    nc.compile()
    return nc


# ==============================================================
# Host side: shard inputs, run, gather output
# ==============================================================

def _prep(inputs):
    """Build per-core input maps (numpy only: layout/sharding/index prep)."""
    emb = np.asarray(inputs["emb"], np.float32)
    items = np.asarray(inputs["session_items"], np.int32)
    lens = np.asarray(inputs["session_len"], np.int32)
    adj = np.asarray(inputs["session_adj"], np.float32)
    erow = np.asarray(inputs["global_edge_row"], np.int32)
    ecol_g = np.asarray(inputs["global_edge_col"], np.int32)
    ew_g = np.asarray(inputs["global_edge_weight"], np.float32)

    rep = {}
    rep["embf"] = emb
    embb = emb.astype(ml_dtypes.bfloat16)
    rep["posemb"] = np.asarray(inputs["pos_emb"], np.float32)
    rep["idn"] = np.eye(H, dtype=np.float32)
    rep["idnb"] = np.eye(H, dtype=ml_dtypes.bfloat16)
    rep["blockdiag"] = np.kron(np.eye(NH, dtype=np.float32),
                               np.ones((H // NH, 1), np.float32))
    rep["w_lin_inT"] = np.ascontiguousarray(np.asarray(inputs["lin_in_W"], np.float32).T)
    rep["w_lin_outT"] = np.ascontiguousarray(np.asarray(inputs["lin_out_W"], np.float32).T)
    rep["b_lin_in"] = np.asarray(inputs["lin_in_b"], np.float32).reshape(H, 1)
    rep["b_lin_out"] = np.asarray(inputs["lin_out_b"], np.float32).reshape(H, 1)
    rep["w_ihT"] = np.ascontiguousarray(np.asarray(inputs["w_ih"], np.float32).T)
    rep["w_hhT"] = np.ascontiguousarray(np.asarray(inputs["w_hh"], np.float32).T)
    rep["b_ih"] = np.asarray(inputs["b_ih"], np.float32).reshape(3 * H, 1)
    rep["b_hh"] = np.asarray(inputs["b_hh"], np.float32).reshape(3 * H, 1)
    ipw = np.asarray(inputs["in_proj_w"], np.float32).copy()
    ipb = np.asarray(inputs["in_proj_b"], np.float32).copy()
    scale = 1.0 / math.sqrt(H // NH)
    ipw[:H] *= scale
    ipb[:H] *= scale
    rep["in_projT"] = np.ascontiguousarray(ipw.T)
    rep["in_projb"] = ipb.reshape(3 * H, 1)
    rep["out_projT"] = np.ascontiguousarray(np.asarray(inputs["out_proj_w"], np.float32).T)
    rep["out_projb"] = np.asarray(inputs["out_proj_b"], np.float32).reshape(H, 1)
    rep["gWTb"] = np.ascontiguousarray(
        np.asarray(inputs["gW"], np.float32).T).astype(ml_dtypes.bfloat16)
    rep["gb"] = np.asarray(inputs["gb"], np.float32).reshape(H, 1)
    rep["w3b"] = np.asarray(inputs["w3_W"], np.float32).astype(ml_dtypes.bfloat16)
    rep["wtTb"] = np.ascontiguousarray(
        np.asarray(inputs["w_target_W"], np.float32).T).astype(ml_dtypes.bfloat16)

    # packed session-membership matrix S [128, NCHP, B]
    offs = np.zeros(B + 1, np.int64)
    np.cumsum(lens, out=offs[1:])
    NVP = int(offs[-1])
    NCHP = (NVP + H - 1) // H
    S = np.zeros((NCHP * H, B), np.float32)
    for b in range(B):
        S[offs[b]:offs[b + 1], b] = 1.0
    rep["ssel"] = np.ascontiguousarray(
        S.reshape(NCHP, H, B).transpose(1, 0, 2)).astype(ml_dtypes.bfloat16)

    # --- global edges: sort by row, shard by vocab range, window-pack ---
    order = np.argsort(erow, kind="stable")
    erow_s, ecol_s, ew_s = erow[order], ecol_g[order], ew_g[order]
    nwin_tot = NC * NWIN
    win_id = erow_s // WIN
    counts = np.bincount(win_id, minlength=nwin_tot)
    T = max(1, int(math.ceil(counts.max() / H)))
    starts = np.zeros(nwin_tot + 1, np.int64)
    np.cumsum(counts, out=starts[1:])

    cand_full = np.zeros((NPAD, H), np.float32)
    cand_full[:NIT - 1] = emb[1:]

    per_core = []
    for c in range(NC):
        NT = NWIN * T
        ec = np.zeros((NT * H,), np.int32)
        er = np.full((NT * H,), 300, np.int64)
        evw = np.zeros((NT * H,), np.float32)
        for w in range(NWIN):
            gw = c * NWIN + w
            s, e = starts[gw], starts[gw + 1]
            n = e - s
            ec[w * T * H: w * T * H + n] = ecol_s[s:e]
            er[w * T * H: w * T * H + n] = erow_s[s:e] - gw * WIN
            evw[w * T * H: w * T * H + n] = ew_s[s:e]
        # [NT*H] -> [H, NT]: tile j, partition p <- j*H + p
        ec2 = ec.reshape(NT, H).T
        er2 = np.ascontiguousarray(er.reshape(NT, H).T)
        ev2 = np.ascontiguousarray(evw.reshape(NT, H).T)
        bsl = slice(c * BLOC, (c + 1) * BLOC)
        it_loc = items[bsl]                      # [8, 50]
        len_loc = lens[bsl]
        pos_idx = np.arange(L)[None, :]
        rev = len_loc[:, None] - 1 - pos_idx
        rev = np.where(it_loc == 0, 0, rev).astype(np.int32)
        pad = (it_loc == 0)

        itemsx = np.zeros((512, 1), np.int32)
        itemsx[:RL, 0] = it_loc.reshape(-1)
        revx = np.zeros((512, 1), np.int32)
        revx[:RL, 0] = rev.reshape(-1)
        attmask = np.where(pad, -1e9, 0.0).astype(np.float32).reshape(1, RL)
        lastsel = np.zeros((BLOC, L), np.float32)
        lastsel[np.arange(BLOC), len_loc - 1] = 1.0

        m = dict(rep)
        m["adjT"] = np.ascontiguousarray(adj[bsl].transpose(0, 2, 1))
        m["itemsx"] = itemsx
        m["revx"] = revx
        m["attmaskr"] = np.broadcast_to(attmask, (NH, RL)).copy()
        m["lastselr"] = np.broadcast_to(lastsel.reshape(1, RL), (H, RL)).copy()
        m["candTb"] = np.ascontiguousarray(
            cand_full[c * NS:(c + 1) * NS].T).astype(ml_dtypes.bfloat16)
        m["eemb"] = np.ascontiguousarray(embb[ec2])
        swa = np.zeros((H, NT, WIN), np.float32)
        swa[np.arange(H)[:, None], np.arange(NT)[None, :],
            np.minimum(er2, WIN - 1)] = ev2
        m["sww"] = swa.astype(ml_dtypes.bfloat16)
        per_core.append(m)
    return per_core, T


def kernel(_trace=False, **inputs):
    in_maps, T = _prep(inputs)
    lens = tuple(int(x) for x in np.asarray(inputs["session_len"], np.int64))
    key = (T, lens)
    if key not in _NC_CACHE:
        _NC_CACHE[key] = build_nc(T, lens)
    nc = _NC_CACHE[key]
    res = run_bass_kernel_spmd(nc, in_maps, core_ids=list(range(NC)),
                               trace=_trace)
    scores = np.concatenate(
        [res.results[c]["scores"] for c in range(NC)], axis=1)[:, :NIT - 1]
    if _trace:
        return scores, res
    return scores


# revision 23
# speedup vs baseline: 1.0981x; 1.0981x over previous
"""Trainium2 Bass kernel for GCE-TAGNN session recommendation model.

Strategy (v2):
  - Vocab axis (10240 = 8*1280) sharded across 8 cores for the global sparse
    aggregation and the candidate transforms; session path data-parallel
    (8 sessions/core); `final` all-gathered in bf16 so every core runs the
    target attention for the full batch over its candidate shard.
  - Global aggregation: edges pre-sorted by row into 128-row windows; the
    one-hot scatter matrices (one-hot(row_rel) * w) are built on the host and
    streamed as bf16, so phase A is pure DMA + chained bf16 matmuls.
  - Target attention in [session-position, candidate] layout:
      ts[bl, n] = final[bl]. (W_t cand[n]),  E = exp(ts),  g[bl, n] = final.d1
      scores[b, n] = (S^T (E*g))[b,n] / (S^T E)[b,n] + last.d2 + sglob.d3
    where S[bl, b] is a host-built 0/1 matrix marking valid (non-pad)
    positions of session b.  Both softmax reductions are matmuls, so the
    inner loop is 4 matmuls + 1 exp + 1 multiply per 128-position chunk,
    software-pipelined to keep the PE at full clock.
"""

import sys

sys.path.insert(0, "/opt/trn_rl_repo")

import math

import ml_dtypes
import numpy as np

import concourse.bass as bass
import concourse.mybir as mybir
import concourse.tile as tile
from concourse import bacc
from concourse.bass import IndirectOffsetOnAxis
from concourse.bass_utils import run_bass_kernel_spmd

F32 = mybir.dt.float32
F32R = mybir.dt.float32r
BF16 = mybir.dt.bfloat16
I32 = mybir.dt.int32
AX = mybir.AxisListType
ALU = mybir.AluOpType
ACT = mybir.ActivationFunctionType

NC = 8          # cores
B = 64          # batch
L = 50          # session length
H = 128         # hidden
NH = 8          # heads
NIT = 10000     # item vocab
NPAD = NC * 1280  # padded vocab for candidate sharding
NS = 1280       # candidate shard per core
BLOC = B // NC  # sessions per core
RL = BLOC * L   # 400 rows per core
BL = B * L      # 3200 positions total
NCH = BL // H   # 25 position chunks
WIN = 128       # agg row window
NWIN = NS // WIN  # 10 windows per core
NSPL = [(0, 512), (512, 512), (1024, 256)]  # candidate splits (PSUM banks)


def _f32r(ap):
    return ap


_NC_CACHE = {}


def build_nc(T, lens):
    """Build the single-NEFF SPMD program. T = edge tiles per window;
    lens = per-session valid lengths (compile-baked packing offsets)."""
    offs = [0]
    for lb in lens:
        offs.append(offs[-1] + int(lb))
    NVP = offs[-1]                      # total valid positions
    NCHP = (NVP + H - 1) // H           # packed chunks
    BLP = NCHP * H
    nc = bacc.Bacc(None, target_bir_lowering=False)

    def inp(name, shape, dtype=F32):
        return nc.dram_tensor(name, shape, dtype, kind="ExternalInput")

    # ---- replicated weights/constants ----
    embf = inp("embf", [NIT, H])
    posemb = inp("posemb", [200, H])
    idn = inp("idn", [H, H])
    idnb = inp("idnb", [H, H], BF16)
    blockdiag = inp("blockdiag", [H, NH])
    w_lin_inT = inp("w_lin_inT", [H, H])
    w_lin_outT = inp("w_lin_outT", [H, H])
    b_lin_in = inp("b_lin_in", [H, 1])
    b_lin_out = inp("b_lin_out", [H, 1])
    w_ihT = inp("w_ihT", [2 * H, 3 * H])
    w_hhT = inp("w_hhT", [H, 3 * H])
    b_ih = inp("b_ih", [3 * H, 1])
    b_hh = inp("b_hh", [3 * H, 1])
    in_projT = inp("in_projT", [H, 3 * H])
    in_projb = inp("in_projb", [3 * H, 1])
    out_projT = inp("out_projT", [H, H])
    out_projb = inp("out_projb", [H, 1])
    gWTb = inp("gWTb", [H, H], BF16)
    gb = inp("gb", [H, 1])
    w3b = inp("w3b", [H, 3 * H], BF16)
    wtTb = inp("wtTb", [H, H], BF16)
    ssel = inp("ssel", [H, NCHP, B], BF16)
    # ---- per-core ----
    adjT = inp("adjT", [BLOC, L, L])
    itemsx = inp("itemsx", [512, 1], I32)
    revx = inp("revx", [512, 1], I32)
    attmaskr = inp("attmaskr", [NH, RL])
    lastselr = inp("lastselr", [H, RL])
    candTb = inp("candTb", [H, NS], BF16)
    eemb = inp("eemb", [H, NWIN * T, H], BF16)
    sww = inp("sww", [H, NWIN * T, WIN], BF16)

    scores_out = nc.dram_tensor("scores", [B, NS], F32, kind="ExternalOutput")

    with tile.TileContext(nc) as tc:
        with (
            tc.tile_pool(name="cst", bufs=1) as cst,
            tc.tile_pool(name="wk", bufs=3) as wk,
            tc.tile_pool(name="pp", bufs=8, space="PSUM") as pp,
            tc.tile_pool(name="dr", bufs=1, space="DRAM") as dr,
        ):
            def psum(shape, tag="ts", nbuf=2, dtype=F32):
                return pp.tile(shape, dtype, tag=tag, name=tag, bufs=nbuf)

            # ---------- load constants into SBUF ----------
            def load(name, src, shape=None, dtype=F32):
                t = cst.tile(shape if shape is not None else src.shape, dtype, name=name)
                nc.sync.dma_start(t[:], src[:])
                return t

            # small consts first (they gate early compute); big slabs after
            NT = NWIN * T
            items_sb = cst.tile([H, 4], I32, name="items_sb")
            nc.sync.dma_start(items_sb[:], itemsx.rearrange("(t p) o -> p (t o)", p=H))
            rev_sb = cst.tile([H, 4], I32, name="rev_sb")
            nc.sync.dma_start(rev_sb[:], revx.rearrange("(t p) o -> p (t o)", p=H))
            gWT_sb = load("gWT_sb", gWTb, dtype=BF16)
            gb_sb = load("gb_sb", gb)
            idnb_sb = load("idnb_sb", idnb, dtype=BF16)
            candT_sb = load("candT_sb", candTb, dtype=BF16)
            w3_sb = load("w3_sb", w3b, dtype=BF16)
            wtT_sb = load("wtT_sb", wtTb, dtype=BF16)
            idn_sb = load("idn_sb", idn)
            eemb_sb = cst.tile([H, NT, H], BF16, name="eemb_sb")
            sww_sb = cst.tile([H, NT, WIN], BF16, name="sww_sb")
            bnd = [round(k * NT / 8) for k in range(9)]
            for k in range(8):
                s, e = bnd[k], bnd[k + 1]
                nc.sync.dma_start(eemb_sb[:, s:e, :], eemb[:, s:e, :])
                nc.sync.dma_start(sww_sb[:, s:e, :], sww[:, s:e, :])
            bd_sb = load("bd_sb", blockdiag)
            linT_sb = load("linT_sb", w_lin_inT)
            loutT_sb = load("loutT_sb", w_lin_outT)
            blin_sb = load("blin_sb", b_lin_in)
            blout_sb = load("blout_sb", b_lin_out)
            wih_sb = cst.tile([H, 2, 3 * H], F32, name="wih_sb")
            nc.sync.dma_start(wih_sb[:], w_ihT.rearrange("(a p) j -> p a j", p=H))
            whh_sb = load("whh_sb", w_hhT)
            bih_sb = cst.tile([H, 3], F32, name="bih_sb")
            bhh_sb = cst.tile([H, 3], F32, name="bhh_sb")
            nc.sync.dma_start(bih_sb[:], b_ih.rearrange("(g p) o -> p (g o)", p=H))
            nc.sync.dma_start(bhh_sb[:], b_hh.rearrange("(g p) o -> p (g o)", p=H))
            prjT_sb = load("prjT_sb", in_projT)
            prjb_sb = cst.tile([H, 3], F32, name="prjb_sb")
            nc.sync.dma_start(prjb_sb[:], in_projb.rearrange("(g p) o -> p (g o)", p=H))
            oprjT_sb = load("oprjT_sb", out_projT)
            oprjb_sb = load("oprjb_sb", out_projb)
            ssel_sb = load("ssel_sb", ssel, dtype=BF16)
            am_sb = load("am_sb", attmaskr)
            ls_sb = load("ls_sb", lastselr)

            # DRAM bounce buffers for collectives
            warm_sh = dr.tile([128, 2], BF16, name="warm_sh")
            warm_fl = dr.tile([NC * 128, 2], BF16, addr_space="Shared",
                              name="warm_fl")
            nc.gpsimd.collective_compute(
                "AllGather", ALU.bypass, replica_groups=[list(range(NC))],
                ins=[warm_sh[:].opt()], outs=[warm_fl[:].opt()])
            hg_shard = dr.tile([NS, H], BF16, name="hg_shard")
            hg_full = dr.tile([NC * NS, H], BF16, addr_space="Shared", name="hg_full")
            f_shard = dr.tile([H, RL], BF16, name="f_shard")
            f_full = dr.tile([NC * H, RL], BF16, addr_space="Shared", name="f_full")
            ls_shard = dr.tile([H, 2 * NH], BF16, name="ls_shard")
            ls_full = dr.tile([NC * H, 2 * NH], BF16, addr_space="Shared",
                              name="ls_full")

            # issue h0/pos gathers early (gpsimd is idle during phase A)
            gh0 = cst.tile([H, 4, H], F32, name="gh0")
            gpo = cst.tile([H, 4, H], F32, name="gpo")
            for t in range(4):
                nc.gpsimd.indirect_dma_start(
                    out=gh0[:, t, :], out_offset=None, in_=embf[:, :],
                    in_offset=IndirectOffsetOnAxis(ap=items_sb[:, t:t + 1], axis=0))
                nc.gpsimd.indirect_dma_start(
                    out=gpo[:, t, :], out_offset=None, in_=posemb[:, :],
                    in_offset=IndirectOffsetOnAxis(ap=rev_sb[:, t:t + 1], axis=0))

            # =======================================================
            # Phase A: global GNN aggregation (vocab shard, 10 windows)
            # agg[h, r] = sum_edges w * emb[col, h] via one-hot matmuls
            # =======================================================
            aggT = cst.tile([H, NS], BF16, name="aggT")
            for w in range(NWIN):
                agg_ps = psum([H, WIN], tag="ts")
                for t in range(T):
                    j = w * T + t
                    nc.tensor.matmul(agg_ps[:], eemb_sb[:, j, :], sww_sb[:, j, :],
                                     start=(t == 0), stop=(t == T - 1))
                nc.vector.tensor_copy(aggT[:, w * WIN:(w + 1) * WIN], agg_ps[:])
            # hgT = relu(gW @ agg + gb) in bf16
            hgT = cst.tile([H, NS], BF16, name="hgT")
            for off, w in NSPL:
                ps = psum([H, w], tag="gg")
                nc.tensor.matmul(ps[:], gWT_sb[:], aggT[:, off:off + w])
                nc.scalar.activation(hgT[:, off:off + w], ps[:], ACT.Relu,
                                     bias=gb_sb[:, :1])
            # transpose to row-major [1280, 128] and store for all-gather
            hg_rm = cst.tile([H, NWIN, H], BF16, name="hg_rm")
            for k in range(NWIN):
                ps_b = psum([H, H], tag="num", dtype=BF16)
                nc.tensor.transpose(ps_b[:], hgT[:, k * H:(k + 1) * H], idnb_sb[:])
                nc.vector.tensor_copy(hg_rm[:, k, :], ps_b[:])
            nc.sync.dma_start(hg_shard.rearrange("(k p) h -> p k h", p=H), hg_rm[:])
            nc.gpsimd.collective_compute(
                "AllGather", ALU.bypass, replica_groups=[list(range(NC))],
                ins=[hg_shard[:].opt()], outs=[hg_full[:].opt()])

            # =======================================================
            # Phase C: candidate transforms (runs in AG1's shadow)
            # cT[j][r, n] = d[n, j*128+r] with d = cand @ w3_W; trT = W_t@cand^T
            # =======================================================
            cT = [cst.tile([H, NS], BF16, name=f"c{j}T") for j in range(3)]
            trT = cst.tile([H, NS], BF16, name="trT")
            for j in range(3):
                for off, w in NSPL:
                    ps = psum([H, w], tag="ts")
                    nc.tensor.matmul(
                        ps[:], w3_sb[:, j * H:(j + 1) * H],
                        candT_sb[:, off:off + w])
                    nc.vector.tensor_copy(cT[j][:, off:off + w], ps[:])
            for off, w in NSPL:
                ps = psum([H, w], tag="gg")
                nc.tensor.matmul(ps[:], wtT_sb[:], candT_sb[:, off:off + w])
                nc.vector.tensor_copy(trT[:, off:off + w], ps[:])

            # =======================================================
            # Phase B: session path (8 local sessions)
            # =======================================================
            def gather_T(dst, table, idx_sb, tag, dtype=F32):
                """gather rows table[idx] -> transpose -> dst [128, 512]."""
                for t in range(4):
                    g = wk.tile([H, H], dtype, tag=tag)
                    nc.gpsimd.indirect_dma_start(
                        out=g[:], out_offset=None, in_=table[:, :],
                        in_offset=IndirectOffsetOnAxis(ap=idx_sb[:, t:t + 1], axis=0))
                    if dtype == BF16:
                        ps_g2 = psum([H, H], tag="num", dtype=BF16)
                        nc.tensor.transpose(ps_g2[:], g[:], idnb_sb[:])
                        nc.vector.tensor_copy(dst[:, t * H:(t + 1) * H], ps_g2[:])
                    else:
                        ps = psum([H, H], tag="den")
                        nc.tensor.transpose(ps[:], g[:], idn_sb[:])
                        nc.vector.tensor_copy(dst[:, t * H:(t + 1) * H], ps[:])

            def transpose_T(dst, gsrc):
                for t in range(4):
                    ps = psum([H, H], tag="den")
                    nc.tensor.transpose(ps[:], gsrc[:, t, :], idn_sb[:])
                    nc.vector.tensor_copy(dst[:, t * H:(t + 1) * H], ps[:])

            h0T = cst.tile([H, 512], F32, name="h0T")
            transpose_T(h0T, gh0)

            # Y = lin(h);  inp = adj @ Y   (per session)
            yinT = cst.tile([H, RL], F32, name="yinT")
            youtT = cst.tile([H, RL], F32, name="youtT")
            ps = psum([H, RL], tag="ts")
            nc.tensor.matmul(ps[:], _f32r(linT_sb[:]), _f32r(h0T[:, :RL]))
            nc.vector.tensor_scalar_add(yinT[:], ps[:], blin_sb[:, :1])
            ps = psum([H, RL], tag="gg")
            nc.tensor.matmul(ps[:], _f32r(loutT_sb[:]), _f32r(h0T[:, :RL]))
            nc.vector.tensor_scalar_add(youtT[:], ps[:], blout_sb[:, :1])

            iinT = cst.tile([H, RL], F32, name="iinT")
            ioutT = cst.tile([H, RL], F32, name="ioutT")
            for b in range(BLOC):
                at = wk.tile([L, L], F32, tag="at")
                nc.sync.dma_start(at[:], adjT[b])
                for yT, dst in ((yinT, iinT), (youtT, ioutT)):
                    ps_t = psum([L, H], tag="den")
                    nc.tensor.transpose(ps_t[:], yT[:, b * L:(b + 1) * L], idn_sb[:])
                    yb = wk.tile([L, H], F32, tag="yb")
                    nc.vector.tensor_copy(yb[:], ps_t[:])
                    ps_i = psum([H, L], tag="num")
                    nc.tensor.matmul(ps_i[:], yb[:], at[:])
                    nc.vector.tensor_copy(dst[:, b * L:(b + 1) * L], ps_i[:])

            # GRU cell (feature-major)
            combR = cst.tile([H, 2], F32, name="combR")
            nc.vector.tensor_add(combR[:, 0:1], bih_sb[:, 0:1], bhh_sb[:, 0:1])
            nc.vector.tensor_add(combR[:, 1:2], bih_sb[:, 1:2], bhh_sb[:, 1:2])
            gates = []
            for g in range(2):  # r, z
                ps_g = psum([H, RL], tag="ts" if g == 0 else "gg")
                nc.tensor.matmul(ps_g[:], _f32r(wih_sb[:, 0, g * H:(g + 1) * H]),
                                 _f32r(iinT[:]), start=True, stop=False)
                nc.tensor.matmul(ps_g[:], _f32r(wih_sb[:, 1, g * H:(g + 1) * H]),
                                 _f32r(ioutT[:]), start=False, stop=False)
                nc.tensor.matmul(ps_g[:], _f32r(whh_sb[:, g * H:(g + 1) * H]),
                                 _f32r(h0T[:, :RL]), start=False, stop=True)
                gt = cst.tile([H, RL], F32, name=f"gate{g}")
                nc.scalar.activation(gt[:], ps_g[:], ACT.Sigmoid, bias=combR[:, g:g + 1])
                gates.append(gt)
            rT, zT = gates
            ps_in = psum([H, RL], tag="ts")
            nc.tensor.matmul(ps_in[:], _f32r(wih_sb[:, 0, 2 * H:3 * H]), _f32r(iinT[:]),
                             start=True, stop=False)
            nc.tensor.matmul(ps_in[:], _f32r(wih_sb[:, 1, 2 * H:3 * H]), _f32r(ioutT[:]),
                             start=False, stop=True)
            ps_hn = psum([H, RL], tag="gg")
            nc.tensor.matmul(ps_hn[:], _f32r(whh_sb[:, 2 * H:3 * H]), _f32r(h0T[:, :RL]))
            rhn = cst.tile([H, RL], F32, name="rhn")
            nc.vector.scalar_tensor_tensor(
                out=rhn[:], in0=ps_hn[:], scalar=bhh_sb[:, 2:3], in1=rT[:],
                op0=ALU.add, op1=ALU.mult)
            tmp_n = cst.tile([H, RL], F32, name="tmp_n")
            nc.vector.tensor_add(tmp_n[:], ps_in[:], rhn[:])
            nT = cst.tile([H, RL], F32, name="nT")
            nc.scalar.activation(nT[:], tmp_n[:], ACT.Tanh, bias=bih_sb[:, 2:3])
            diff = cst.tile([H, RL], F32, name="diff")
            nc.vector.tensor_sub(diff[:], h0T[:, :RL], nT[:])
            zd = cst.tile([H, RL], F32, name="zd")
            nc.vector.tensor_mul(zd[:], zT[:], diff[:])
            h1T = cst.tile([H, RL], F32, name="h1T")
            nc.vector.tensor_add(h1T[:], nT[:], zd[:])

            # rich = hg[items] + h1; final = rich + pos_emb[rev]
            sgT = cst.tile([H, 512], BF16, name="sgT")
            gather_T(sgT, hg_full, items_sb, "gsg", dtype=BF16)
            poT = cst.tile([H, 512], F32, name="poT")
            transpose_T(poT, gpo)
            richT = cst.tile([H, RL], F32, name="richT")
            nc.vector.tensor_add(richT[:], h1T[:], sgT[:, :RL])
            # ship final (bf16) for all-gather ASAP; rest overlaps the collective
            finB = cst.tile([H, RL], BF16, name="finB")
            nc.vector.tensor_add(finB[:], richT[:], poT[:, :RL])
            nc.sync.dma_start(f_shard[:], finB[:])
            nc.gpsimd.collective_compute(
                "AllGather", ALU.bypass, replica_groups=[list(range(NC))],
                ins=[f_shard[:].opt()], outs=[f_full[:].opt()])
            finT = cst.tile([H, RL], F32, name="finT")
            nc.vector.tensor_add(finT[:], richT[:], poT[:, :RL])


            # last[b] = final[b, len_b - 1]  (one-hot selection + reduce)
            lsel = cst.tile([H, RL], F32, name="lsel")
            nc.vector.tensor_mul(lsel[:], finT[:], ls_sb[:])
            lastT = cst.tile([H, NH], F32, name="lastT")
            nc.vector.reduce_sum(lastT[:], lsel[:].rearrange("p (b l) -> p b l", b=BLOC),
                                 axis=AX.X)
            # ---- multi-head attention (q = last, kv = final) ----
            qT = cst.tile([H, NH], F32, name="qT")
            ps_q = psum([H, NH], tag="den")
            nc.tensor.matmul(ps_q[:], _f32r(prjT_sb[:, 0:H]), _f32r(lastT[:]))
            nc.vector.tensor_scalar_add(qT[:], ps_q[:], prjb_sb[:, 0:1])
            kT = cst.tile([H, RL], F32, name="kT")
            ps_k = psum([H, RL], tag="ts")
            nc.tensor.matmul(ps_k[:], _f32r(prjT_sb[:, H:2 * H]), _f32r(finT[:]))
            nc.vector.tensor_scalar_add(kT[:], ps_k[:], prjb_sb[:, 1:2])
            vT = cst.tile([H, RL], F32, name="vT")
            ps_v = psum([H, RL], tag="gg")
            nc.tensor.matmul(ps_v[:], _f32r(prjT_sb[:, 2 * H:3 * H]), _f32r(finT[:]))
            nc.vector.tensor_scalar_add(vT[:], ps_v[:], prjb_sb[:, 2:3])

            ctxT = cst.tile([H, NH], F32, name="ctxT")
            for b in range(BLOC):
                qb = wk.tile([H, NH], F32, tag="qb")
                nc.vector.tensor_mul(qb[:], qT[:, b:b + 1].to_broadcast([H, NH]), bd_sb[:])
                ps_a = psum([NH, L], tag="ts")
                nc.tensor.matmul(ps_a[:], qb[:], kT[:, b * L:(b + 1) * L])
                attm = wk.tile([NH, L], F32, tag="attm")
                nc.vector.tensor_add(attm[:], ps_a[:], am_sb[:, b * L:(b + 1) * L])
                negmax = wk.tile([NH, 1], F32, tag="negmax")
                nc.vector.tensor_reduce(negmax[:], attm[:], axis=AX.X, op=ALU.max,
                                        negate=True)
                attE = wk.tile([NH, L], F32, tag="attE")
                den_a = wk.tile([NH, 1], F32, tag="den_a")
                nc.scalar.activation(attE[:], attm[:], ACT.Exp, bias=negmax[:, :1],
                                     accum_out=den_a[:, :1])
                rec_a = wk.tile([NH, 1], F32, tag="rec_a")
                nc.vector.reciprocal(rec_a[:], den_a[:])
                attw = wk.tile([NH, L], F32, tag="attw")
                nc.vector.tensor_scalar_mul(attw[:], attE[:], rec_a[:, :1])
                ps_wt = psum([L, NH], tag="gg")
                nc.tensor.transpose(ps_wt[:], attw[:], idn_sb[:NH, :NH])
                awT = wk.tile([L, NH], F32, tag="awT")
                nc.vector.tensor_copy(awT[:], ps_wt[:])
                ps_vt = psum([L, H], tag="den")
                nc.tensor.transpose(ps_vt[:], vT[:, b * L:(b + 1) * L], idn_sb[:])
                vb = wk.tile([L, H], F32, tag="vb")
                nc.vector.tensor_copy(vb[:], ps_vt[:])
                ps_o = psum([H, NH], tag="num")
                nc.tensor.matmul(ps_o[:], vb[:], awT[:])
                o2 = wk.tile([H, NH], F32, tag="o2")
                nc.vector.tensor_mul(o2[:], ps_o[:], bd_sb[:])
                nc.vector.reduce_sum(ctxT[:, b:b + 1], o2[:], axis=AX.X)

            sgloT = cst.tile([H, NH], F32, name="sgloT")
            ps_sg = psum([H, NH], tag="den")
            nc.tensor.matmul(ps_sg[:], _f32r(oprjT_sb[:]), _f32r(ctxT[:]))
            nc.vector.tensor_scalar_add(sgloT[:], ps_sg[:], oprjb_sb[:, :1])

            lsB = cst.tile([H, 2 * NH], BF16, name="lsB")
            nc.vector.tensor_copy(lsB[:, 0:NH], lastT[:])
            nc.vector.tensor_copy(lsB[:, NH:2 * NH], sgloT[:])
            nc.sync.dma_start(ls_shard[:], lsB[:])
            nc.gpsimd.collective_compute(
                "AllGather", ALU.bypass, replica_groups=[list(range(NC))],
                ins=[ls_shard[:].opt()], outs=[ls_full[:].opt()])

            fullT = cst.tile([H, BL], BF16, name="fullT")
            ffv = f_full.rearrange("(c p) r -> p c r", p=H)
            for k in range(4):
                nc.sync.dma_start(
                    fullT[:].rearrange("p (c r) -> p c r", c=NC)[:, 2 * k:2 * k + 2, :],
                    ffv[:, 2 * k:2 * k + 2, :])
            # pack valid session prefixes contiguously (offsets compile-baked)
            fullP = cst.tile([H, BLP], BF16, name="fullP")
            if BLP > NVP:
                nc.vector.memset(fullP[:, NVP:BLP], 0)
            for b in range(B):
                nc.vector.tensor_copy(fullP[:, offs[b]:offs[b + 1]],
                                      fullT[:, b * L:b * L + int(lens[b])])

            lsv = ls_full.rearrange("(c p) x -> p c x", p=H)
            lastF = cst.tile([H, B], BF16, name="lastF")
            sglF = cst.tile([H, B], BF16, name="sglF")
            nc.sync.dma_start(lastF[:].rearrange("p (c x) -> p c x", c=NC),
                              lsv[:, :, 0:NH])
            nc.sync.dma_start(sglF[:].rearrange("p (c x) -> p c x", c=NC),
                              lsv[:, :, NH:2 * NH])

            # =======================================================
            # Phase D: target attention, [position, candidate] layout
            # =======================================================
            def d_epilogue(noff, nw, ps_den, ps_num):
                # scores = num/den + last.d2 + sglob.d3
                rec = wk.tile([B, nw], F32, tag="rec", bufs=2)
                nc.vector.reciprocal(rec[:], ps_den[:])
                ps23 = psum([B, nw], tag="ts")
                nc.tensor.matmul(ps23[:], lastF[:], cT[1][:, noff:noff + nw],
                                 start=True, stop=False)
                nc.tensor.matmul(ps23[:], sglF[:], cT[2][:, noff:noff + nw],
                                 start=False, stop=True)
                t1 = wk.tile([B, nw], F32, tag="t1", bufs=2)
                nc.vector.tensor_mul(t1[:], ps_num[:], rec[:])
                outT = wk.tile([B, nw], F32, tag="outT", bufs=2)
                nc.vector.tensor_add(outT[:], t1[:], ps23[:])
                nc.sync.dma_start(scores_out[:, noff:noff + nw], outT[:])

            # --- splits 0+1 fused: one weight-load per chunk for 4 matmuls ---
            W2 = 512
            d0 = psum([B, W2], tag="den")
            n0 = psum([B, W2], tag="num")
            d1 = psum([B, W2], tag="den")
            n1 = psum([B, W2], tag="num")
            pend = []
            for c in range(NCHP):
                fp = fullP[:, c * H:(c + 1) * H]
                ts0 = psum([H, W2], tag="ts")
                nc.tensor.matmul(ts0[:], fp, trT[:, 0:W2])
                ts1 = psum([H, W2], tag="ts")
                nc.tensor.matmul(ts1[:], fp, trT[:, W2:2 * W2])
                g0 = psum([H, W2], tag="gg")
                nc.tensor.matmul(g0[:], fp, cT[0][:, 0:W2])
                g1 = psum([H, W2], tag="gg")
                nc.tensor.matmul(g1[:], fp, cT[0][:, W2:2 * W2])
                if len(pend) >= 2:
                    eA, pA, eB, pB, cP = pend.pop(0)
                    sv = ssel_sb[:, cP, :]
                    nc.tensor.matmul(d0[:], sv, eA[:], start=(cP == 0), stop=False)
                    nc.tensor.matmul(n0[:], sv, pA[:], start=(cP == 0), stop=False)
                    nc.tensor.matmul(d1[:], sv, eB[:], start=(cP == 0), stop=False)
                    nc.tensor.matmul(n1[:], sv, pB[:], start=(cP == 0), stop=False)
                eT0 = wk.tile([H, W2], BF16, tag="eT", bufs=6)
                nc.scalar.activation(eT0[:], ts0[:], ACT.Exp)
                pT0 = wk.tile([H, W2], BF16, tag="pT", bufs=6)
                nc.vector.tensor_mul(pT0[:], eT0[:], g0[:])
                eT1 = wk.tile([H, W2], BF16, tag="eT", bufs=6)
                nc.scalar.activation(eT1[:], ts1[:], ACT.Exp)
                pT1 = wk.tile([H, W2], BF16, tag="pT", bufs=6)
                nc.vector.tensor_mul(pT1[:], eT1[:], g1[:])
                pend.append((eT0, pT0, eT1, pT1, c))
            for eA, pA, eB, pB, cP in pend:
                sv = ssel_sb[:, cP, :]
                st = (cP == NCHP - 1)
                nc.tensor.matmul(d0[:], sv, eA[:], start=False, stop=st)
                nc.tensor.matmul(n0[:], sv, pA[:], start=False, stop=st)
                nc.tensor.matmul(d1[:], sv, eB[:], start=False, stop=st)
                nc.tensor.matmul(n1[:], sv, pB[:], start=False, stop=st)

            d_epilogue(0, W2, d0, n0)

            # --- split 2 (256 wide): den|num merged into one matmul ---
            W3 = 256
            d2 = psum([B, 2 * W3], tag="den")
            pend2 = []
            for c in range(NCHP):
                fp = fullP[:, c * H:(c + 1) * H]
                ts2 = psum([H, W3], tag="ts")
                nc.tensor.matmul(ts2[:], fp, trT[:, 2 * W2:2 * W2 + W3])
                g2 = psum([H, W3], tag="gg")
                nc.tensor.matmul(g2[:], fp, cT[0][:, 2 * W2:2 * W2 + W3])
                if len(pend2) >= 2:
                    ep, cP = pend2.pop(0)
                    nc.tensor.matmul(d2[:], ssel_sb[:, cP, :], ep[:],
                                     start=(cP == 0), stop=False)
                ep2 = wk.tile([H, 2 * W3], BF16, tag="ep2", bufs=4)
                nc.scalar.activation(ep2[:, 0:W3], ts2[:], ACT.Exp)
                nc.vector.tensor_mul(ep2[:, W3:2 * W3], ep2[:, 0:W3], g2[:])
                pend2.append((ep2, c))
            for ep, cP in pend2:
                nc.tensor.matmul(d2[:], ssel_sb[:, cP, :], ep[:],
                                 start=False, stop=(cP == NCHP - 1))

            d_epilogue(W2, W2, d1, n1)
            d_epilogue(2 * W2, W3, d2[:, 0:W3], d2[:, W3:2 * W3])

    nc.compile()
    return nc


# ==============================================================
# Host side: shard inputs, run, gather output
# ==============================================================

def _prep(inputs):
    """Build per-core input maps (numpy only: layout/sharding/index prep)."""
    emb = np.asarray(inputs["emb"], np.float32)
    items = np.asarray(inputs["session_items"], np.int32)
    lens = np.asarray(inputs["session_len"], np.int32)
    adj = np.asarray(inputs["session_adj"], np.float32)
    erow = np.asarray(inputs["global_edge_row"], np.int32)
    ecol_g = np.asarray(inputs["global_edge_col"], np.int32)
    ew_g = np.asarray(inputs["global_edge_weight"], np.float32)

    rep = {}
    rep["embf"] = emb
    embb = emb.astype(ml_dtypes.bfloat16)
    rep["posemb"] = np.asarray(inputs["pos_emb"], np.float32)
    rep["idn"] = np.eye(H, dtype=np.float32)
    rep["idnb"] = np.eye(H, dtype=ml_dtypes.bfloat16)
    rep["blockdiag"] = np.kron(np.eye(NH, dtype=np.float32),
                               np.ones((H // NH, 1), np.float32))
    rep["w_lin_inT"] = np.ascontiguousarray(np.asarray(inputs["lin_in_W"], np.float32).T)
    rep["w_lin_outT"] = np.ascontiguousarray(np.asarray(inputs["lin_out_W"], np.float32).T)
    rep["b_lin_in"] = np.asarray(inputs["lin_in_b"], np.float32).reshape(H, 1)
    rep["b_lin_out"] = np.asarray(inputs["lin_out_b"], np.float32).reshape(H, 1)
    rep["w_ihT"] = np.ascontiguousarray(np.asarray(inputs["w_ih"], np.float32).T)
    rep["w_hhT"] = np.ascontiguousarray(np.asarray(inputs["w_hh"], np.float32).T)
    rep["b_ih"] = np.asarray(inputs["b_ih"], np.float32).reshape(3 * H, 1)
    rep["b_hh"] = np.asarray(inputs["b_hh"], np.float32).reshape(3 * H, 1)
    ipw = np.asarray(inputs["in_proj_w"], np.float32).copy()
    ipb = np.asarray(inputs["in_proj_b"], np.float32).copy()
    scale = 1.0 / math.sqrt(H // NH)
    ipw[:H] *= scale
    ipb[:H] *= scale
    rep["in_projT"] = np.ascontiguousarray(ipw.T)
    rep["in_projb"] = ipb.reshape(3 * H, 1)
    rep["out_projT"] = np.ascontiguousarray(np.asarray(inputs["out_proj_w"], np.float32).T)
    rep["out_projb"] = np.asarray(inputs["out_proj_b"], np.float32).reshape(H, 1)
    rep["gWTb"] = np.ascontiguousarray(
        np.asarray(inputs["gW"], np.float32).T).astype(ml_dtypes.bfloat16)
    rep["gb"] = np.asarray(inputs["gb"], np.float32).reshape(H, 1)
    rep["w3b"] = np.asarray(inputs["w3_W"], np.float32).astype(ml_dtypes.bfloat16)
    rep["wtTb"] = np.ascontiguousarray(
        np.asarray(inputs["w_target_W"], np.float32).T).astype(ml_dtypes.bfloat16)

    # packed session-membership matrix S [128, NCHP, B]
    offs = np.zeros(B + 1, np.int64)
    np.cumsum(lens, out=offs[1:])
    NVP = int(offs[-1])
    NCHP = (NVP + H - 1) // H
    S = np.zeros((NCHP * H, B), np.float32)
    for b in range(B):
        S[offs[b]:offs[b + 1], b] = 1.0
    rep["ssel"] = np.ascontiguousarray(
        S.reshape(NCHP, H, B).transpose(1, 0, 2)).astype(ml_dtypes.bfloat16)

    # --- global edges: sort by row, shard by vocab range, window-pack ---
    order = np.argsort(erow, kind="stable")
    erow_s, ecol_s, ew_s = erow[order], ecol_g[order], ew_g[order]
    nwin_tot = NC * NWIN
    win_id = erow_s // WIN
    counts = np.bincount(win_id, minlength=nwin_tot)
    T = max(1, int(math.ceil(counts.max() / H)))
    starts = np.zeros(nwin_tot + 1, np.int64)
    np.cumsum(counts, out=starts[1:])

    cand_full = np.zeros((NPAD, H), np.float32)
    cand_full[:NIT - 1] = emb[1:]

    per_core = []
    for c in range(NC):
        NT = NWIN * T
        ec = np.zeros((NT * H,), np.int32)
        er = np.full((NT * H,), 300, np.int64)
        evw = np.zeros((NT * H,), np.float32)
        for w in range(NWIN):
            gw = c * NWIN + w
            s, e = starts[gw], starts[gw + 1]
            n = e - s
            ec[w * T * H: w * T * H + n] = ecol_s[s:e]
            er[w * T * H: w * T * H + n] = erow_s[s:e] - gw * WIN
            evw[w * T * H: w * T * H + n] = ew_s[s:e]
        # [NT*H] -> [H, NT]: tile j, partition p <- j*H + p
        ec2 = ec.reshape(NT, H).T
        er2 = np.ascontiguousarray(er.reshape(NT, H).T)
        ev2 = np.ascontiguousarray(evw.reshape(NT, H).T)
        bsl = slice(c * BLOC, (c + 1) * BLOC)
        it_loc = items[bsl]                      # [8, 50]
        len_loc = lens[bsl]
        pos_idx = np.arange(L)[None, :]
        rev = len_loc[:, None] - 1 - pos_idx
        rev = np.where(it_loc == 0, 0, rev).astype(np.int32)
        pad = (it_loc == 0)

        itemsx = np.zeros((512, 1), np.int32)
        itemsx[:RL, 0] = it_loc.reshape(-1)
        revx = np.zeros((512, 1), np.int32)
        revx[:RL, 0] = rev.reshape(-1)
        attmask = np.where(pad, -1e9, 0.0).astype(np.float32).reshape(1, RL)
        lastsel = np.zeros((BLOC, L), np.float32)
        lastsel[np.arange(BLOC), len_loc - 1] = 1.0

        m = dict(rep)
        m["adjT"] = np.ascontiguousarray(adj[bsl].transpose(0, 2, 1))
        m["itemsx"] = itemsx
        m["revx"] = revx
        m["attmaskr"] = np.broadcast_to(attmask, (NH, RL)).copy()
        m["lastselr"] = np.broadcast_to(lastsel.reshape(1, RL), (H, RL)).copy()
        m["candTb"] = np.ascontiguousarray(
            cand_full[c * NS:(c + 1) * NS].T).astype(ml_dtypes.bfloat16)
        m["eemb"] = np.ascontiguousarray(embb[ec2])
        swa = np.zeros((H, NT, WIN), np.float32)
        swa[np.arange(H)[:, None], np.arange(NT)[None, :],
            np.minimum(er2, WIN - 1)] = ev2
        m["sww"] = swa.astype(ml_dtypes.bfloat16)
        per_core.append(m)
    return per_core, T


def kernel(_trace=False, **inputs):
    in_maps, T = _prep(inputs)
    lens = tuple(int(x) for x in np.asarray(inputs["session_len"], np.int64))
    key = (T, lens)
    if key not in _NC_CACHE:
        _NC_CACHE[key] = build_nc(T, lens)
    nc = _NC_CACHE[key]
    res = run_bass_kernel_spmd(nc, in_maps, core_ids=list(range(NC)),
                               trace=_trace)
    scores = np.concatenate(
        [res.results[c]["scores"] for c in range(NC)], axis=1)[:, :NIT - 1]
    if _trace:
        return scores, res
    return scores
